# revision 58
# baseline (speedup 1.0000x reference)
"""Causal self-attention (B=2, T=2048, C=1024, H=16) on 8 trn2 cores.

Sharding (Megatron-style): core i -> batch b = i//4, head group g = i%4
(heads 4g..4g+3, dloc = 256 dims); Wq/Wk/Wv column-parallel, Wp
row-parallel; host sums the 4 partial y contributions per batch.

Per-core kernel, all matmul operands bf16 (PSUM f32):
  - QKV: x in SBUF t-tile-major (tq, c, t), matching the host-prepped
    DRAM layout so every input DMA is contiguous on both sides (strided
    DRAM patterns measured ~70GB/s vs full bandwidth contiguous).
    q^T d-major [128(2h x 64d), T]; v t-major tiles [128 t, 4h x 65]
    with a ones column per head (col 64) so the softmax denominator
    accumulates in the pv matmul.
  - scores S^T per (j q-block of 512, p head-pair, tk k-tile of 128):
    the k stationary is zero-padded to K=128 (kTz[p][hb]: head 2p+hb in
    rows 64hb..64hb+64, zeros elsewhere) because a K=64 matmul streams
    at HALF rate (427ns vs 216ns per 512 cols on HW); the full 128-row
    q^T is the moving data, so each head's scores run at full rate.
    psum [128 kpos, 2x512], causally clipped; exp on Act -> pT bf16;
    triangular 0/1 mask multiply (DVE) on the diagonal subtile only.
  - pv q-major: ctx psum [128 q, 4h x 65] per (j, qsub of 128),
    accumulated over tk with pT subtiles as the stationary (65-col
    streams issue at ~30ns; LDWEIGHTS is fully hidden). Normalization =
    reciprocal + tensor_scalar muls on DVE -> cn bf16.
  - PE transpose (identity matmul) -> ctxT [128 d, 128 t] bf16; output
    projection accumulates the two 128-d slices; y DMA'd per tile.
  - Schedule: 52 warmup matmuls ramp the PE DVFS while input DMAs land
    (x on the sync HWDGE queue, weights on the scalar queue, in
    parallel); a deque of PE work chunks (next QKV block, yproj tiles)
    is pumped between score matmuls; yproj of j0/j1 is deferred into
    the j2/j3 stream where Act (exp) is the bottleneck and PE idles on
    the score-psum WAR.
Measured ~149.5-152us on HW (from 157us baseline); PE-bound: ~118us busy on
~242k matmul columns, Act (exp) ~80us, DVE ~75us.
"""

import sys
from collections import deque

import numpy as np

if "/opt/trn_rl_repo" not in sys.path:
    sys.path.insert(0, "/opt/trn_rl_repo")

B, T, C = 2, 2048, 1024
H, D = 16, 64
HPC = 4            # heads per core
DLOC = HPC * D     # 256 local head dims
NQ = 512           # q block width (j)
NK = 128           # k tile width (tk)
NCSL = C // 128    # 8 contraction slices
NTQ = T // NQ      # 4 q blocks
NT128 = T // 128   # 16 k tiles
N_CORES = 8

_NC_CACHE = None


def _emit(nc, tc, aps, dbg=None):
    from contextlib import ExitStack

    import concourse.mybir as mybir

    f32 = mybir.dt.float32
    bf16 = mybir.dt.bfloat16
    Exp = mybir.ActivationFunctionType.Exp

    xT_d, wqT_d, wkT_d, wvT_d, wpT_d, mask2_d, ident_d, y_d = aps

    def dump(name, ap, pool):
        # debug: copy an SBUF/PSUM AP to a DRAM output
        if dbg is None or name not in dbg:
            return
        shape = [ap.shape[0], ap.free_size()]
        t = pool.tile(shape, f32, name=f"dbg_{name}_t")
        nc.vector.tensor_copy(t, ap)
        nc.sync.dma_start(out=dbg[name], in_=t)

    mm = nc.tensor.matmul

    with ExitStack() as top:
        const = top.enter_context(tc.tile_pool(name="const", bufs=1))
        stage = top.enter_context(tc.tile_pool(name="stage", bufs=1))
        psum = top.enter_context(tc.tile_pool(name="psum", bufs=1, space="PSUM"))

        # ---------------- persistent SBUF tensors -------------------------
        # xts free layout: (tq, c, t) t-tile-major, matching the host's
        # xr layout exactly so every x DMA is contiguous on both sides
        # (strided DRAM patterns measured 256B lines / ~70GB/s).
        xts = const.tile([128, NCSL * T], bf16, name="xts")       # 32KB/part
        xts_t = xts.rearrange("p (tq c t) -> p tq c t", tq=NT128, c=NCSL)
        wq_sb = const.tile([128, NCSL * DLOC], bf16, name="wq")   # 4KB
        wk_sb = const.tile([128, NCSL * DLOC], bf16, name="wk")
        wv_sb = const.tile([128, NCSL * DLOC], bf16, name="wv")
        wp_sb = const.tile([128, 2 * C], bf16, name="wp")         # 4KB
        mask2 = const.tile([128, 256], bf16, name="mask2")
        ident = const.tile([128, 128], bf16, name="ident")
        qT = [const.tile([128, T], bf16, name=f"qT{p}") for p in range(2)]
        # k stationaries zero-padded to K=128: kTz[p][hb] holds head 2p+hb's
        # kT in rows 64*hb..64*hb+64, zeros elsewhere.  A K=64 matmul
        # streams at HALF rate (measured 427ns vs 216ns per 512 cols), so
        # padding the contraction to 128 and using the full 128-row qT as
        # moving data doubles score throughput for free.
        kTz = [[const.tile([128, T], bf16, name=f"kTz{p}_{hb}")
                for hb in range(2)] for p in range(2)]
        vt = [const.tile([128, HPC * 65], bf16, name=f"v{t}") for t in range(NT128)]
        # exp'd score tiles pT[p][tk]: [128 kpos, 2 heads x 512 q] bf16
        pT = [[const.tile([128, 2 * NQ], bf16, name=f"pT{p}_{t}")
               for t in range(NT128)] for p in range(2)]

        # ---------------- DMA loads ---------------------------------------
        # Host sends x/weights already in SBUF layout: every DMA below is
        # a contiguous range on both sides (full-bandwidth, 2KB+ lines).
        # x on the sync queue, weights on the scalar queue: the two HWDGE
        # queues run concurrently, halving the serial input-DMA latency
        # that gates the first scores (Act is idle this early, so the
        # trigger cost on its queue is free).
        TQW = NCSL * 128   # 1024 free elems per t-tile

        def dma_x(n):
            cs = slice(4 * TQW * n, 4 * TQW * (n + 1))
            nc.sync.dma_start(out=xts[:, cs], in_=xT_d[:, cs])

        def dma_x_qtr(n, h):
            cs = slice(TQW * (4 * n + h), TQW * (4 * n + h + 1))
            nc.sync.dma_start(out=xts[:, cs], in_=xT_d[:, cs])

        def dma_w(dst, src):
            nc.scalar.dma_start(out=dst, in_=src)

        dma_w(wq_sb, wqT_d)
        for h in range(4):
            dma_x_qtr(0, h)
        dma_w(wk_sb, wkT_d)
        dma_w(wv_sb, wvT_d)
        dma_x(1)
        nc.scalar.dma_start(out=mask2, in_=mask2_d)
        nc.scalar.dma_start(out=ident, in_=ident_d)
        nc.scalar.dma_start(out=wp_sb, in_=wpT_d)
        dma_x(2)
        dma_x(3)

        # ones columns in v tiles (col 64 of each 65-wide head block)
        for t in range(NT128):
            nc.gpsimd.memset(
                vt[t].rearrange("p (h e) -> p h e", e=65)[:, :, 64:65], 1.0)
        # zero halves of the padded k stationaries (once, up front)
        for p in range(2):
            nc.gpsimd.memset(kTz[p][0][64:128, :], 0.0)
            nc.gpsimd.memset(kTz[p][1][0:64, :], 0.0)

        # PE clock warmup: the tensor engine starts at a low p-state and
        # needs continuous work to ramp; burn dummy matmuls on a scratch
        # tile while the first x/w DMAs are in flight so QKV block 0 runs
        # at full clock.
        warm = const.tile([128, 128], bf16, name="warm")
        nc.vector.memset(warm, 0.5)
        wps = psum.tile([128, 128], f32, tag="mm", bufs=2, name="warm")
        for i in range(52):
            mm(wps, warm, warm, start=(i == 0), stop=(i == 51))

        # ---------------- pipelined emission ------------------------------
        # rest_q: next QKV block (gates next j's scores), then yproj tiles,
        # pumped between score emissions to keep PE busy while Act drains
        # exps. pv chains are emitted at the top of each j (they must
        # precede j's exps, which overwrite the pT tiles they read; Act's
        # exp backlog from j-1 keeps it busy meanwhile).
        rest_q = deque()

        def pump(n):
            for _ in range(n):
                if rest_q:
                    rest_q.popleft()[1]()
                else:
                    return

        def xsl(c, lo, hi):
            # lo/hi are 128-aligned t positions; free dims (tq, t)
            return xts_t[:, lo // 128:hi // 128, c, :]

        def qkv_chunks(n, split_qk=False):
            # one chunk per psum chain: q/k (2 m-groups each), v (4 t-tiles)

            def qk_chain(w_sb, which, m, lo=0, hi=NQ):
                def go():
                    ps = psum.tile([128, hi - lo], f32, tag="mm", bufs=2,
                                   name=f"qk{n}{m}{lo}")
                    for c in range(NCSL):
                        mm(ps, w_sb[:, DLOC * c + 128 * m:DLOC * c + 128 * (m + 1)],
                           xsl(c, NQ * n + lo, NQ * n + hi),
                           start=(c == 0), stop=(c == NCSL - 1))
                    cs = slice(NQ * n + lo, NQ * n + hi)
                    if which == "q":
                        nc.vector.tensor_copy(qT[m][:, cs], ps)
                    else:
                        nc.vector.tensor_copy(kTz[m][0][0:64, cs], ps[0:64, :])
                        nc.vector.tensor_copy(kTz[m][1][64:128, cs],
                                              ps[64:128, :])
                return go

            def v_chain(t):
                def go():
                    ps = psum.tile([128, NQ], f32, tag="mm", bufs=2,
                                   name=f"v{t}")
                    pv = ps[:, 0:DLOC]
                    for c in range(NCSL):
                        mm(pv, xsl(c, 128 * t, 128 * (t + 1)),
                           wv_sb[:, DLOC * c:DLOC * (c + 1)],
                           start=(c == 0), stop=(c == NCSL - 1))
                    nc.vector.tensor_copy(
                        vt[t].rearrange("p (h e) -> p h e", e=65)[:, :, 0:64],
                        pv.rearrange("p (h e) -> p h e", e=64))
                return go

            out = []
            if split_qk:
                # quarter-width chains so block-0 QKV starts as soon as
                # wq/wk + the first x quarter DMAs land
                for h in range(4):
                    for m in range(2):
                        out.append((f"q{n}m{m}", qk_chain(wq_sb, "q", m,
                                                          128 * h, 128 * (h + 1))))
                    for m in range(2):
                        out.append((f"k{n}m{m}", qk_chain(wk_sb, "k", m,
                                                          128 * h, 128 * (h + 1))))
            else:
                for m in range(2):
                    out.append((f"q{n}", qk_chain(wq_sb, "q", m)))
                for m in range(2):
                    out.append((f"k{n}", qk_chain(wk_sb, "k", m)))
            for t in range(4 * n, 4 * n + 4):
                out.append((f"v{n}t{t}", v_chain(t)))
            return out

        def scores_exp_mask(j, p, tk):
            di = tk - 4 * j
            lo = 128 * di if di > 0 else 0    # clipped q offset within block
            w = NQ - lo
            ks = slice(NK * tk, NK * (tk + 1))
            qs = slice(NQ * j + lo, NQ * (j + 1))
            s = psum.tile([128, 2 * NQ], f32, tag="sc", bufs=2,
                          name=f"s{j}{p}{tk}")
            mm(s[:, lo:NQ], kTz[p][0][:, ks], qT[p][:, qs],
               start=True, stop=True)
            mm(s[:, NQ + lo:2 * NQ], kTz[p][1][:, ks], qT[p][:, qs],
               start=True, stop=True)
            sv = s.rearrange("p (b q) -> p b q", b=2)[:, :, lo:NQ]
            pv_ = pT[p][tk].rearrange("p (b q) -> p b q", b=2)[:, :, lo:NQ]
            nc.scalar.activation(pv_, sv, Exp)
            if di >= 0:
                dsl = pT[p][tk].rearrange("p (b q) -> p b q", b=2)[
                    :, :, 128 * di:128 * (di + 1)]
                nc.vector.tensor_mul(
                    dsl, dsl, mask2.rearrange("p (b q) -> p b q", b=2))

        def pv_chunk(j, qsub):
            # ctx accumulation for q subtile [128] over tk chain + norm +
            # transpose + ctxT copies
            ntk = 4 * j + qsub + 1

            def go():
                cx = psum.tile([128, HPC * 65], f32, tag="cx", bufs=2,
                               name=f"cx{j}{qsub}")
                for tk in range(ntk):
                    for p in range(2):
                        for hb in range(2):
                            h = 2 * p + hb
                            # one accumulation group for the whole cx bank:
                            # start marks the full 2KB zero region, so only
                            # the first mm starts and only the last stops.
                            mm(cx[:, 65 * h:65 * (h + 1)],
                               pT[p][tk][:, NQ * hb + 128 * qsub:
                                         NQ * hb + 128 * (qsub + 1)],
                               vt[tk][:, 65 * h:65 * (h + 1)],
                               start=(tk == 0 and h == 0),
                               stop=(tk == ntk - 1 and h == 3))
                if j == 0 and qsub == 0:
                    dump("cx00", cx, stage)
                cxv = cx.rearrange("p (h e) -> p h e", e=65)
                rs = stage.tile([128, HPC], f32, tag="rs", bufs=2,
                                name=f"rs{j}{qsub}")
                nc.vector.reciprocal(rs, cxv[:, :, 64:65])
                cn = stage.tile([128, DLOC], bf16, tag="cn", bufs=2,
                                name=f"cn{j}{qsub}")
                for h in range(HPC):
                    nc.vector.tensor_scalar_mul(
                        cn[:, D * h:D * (h + 1)], cxv[:, h, 0:64],
                        rs[:, h:h + 1])
                for p in range(2):
                    tp = psum.tile([128, 128], bf16, tag="mm", bufs=2,
                                   name=f"tp{j}{qsub}{p}")
                    nc.tensor.transpose(
                        tp, cn[:, 128 * p:128 * (p + 1)], ident)
                    ct = stage.tile([128, 128], bf16, tag="ct", bufs=32,
                                    name=f"ct{j}{qsub}{p}")
                    nc.vector.tensor_copy(ct, tp)
                    ctxT[(j, qsub, p)] = ct
                if j == 0 and qsub == 0:
                    dump("cn00", cn, stage)
                    dump("ct000", ctxT[(0, 0, 0)], stage)
            return go

        def yproj_chunk(j, qsub, n):
            tt = 4 * j + qsub

            def go():
                yps = psum.tile([128, NQ], f32, tag="mm", bufs=2,
                                name=f"y{tt}{n}")
                for p in range(2):
                    mm(yps, ctxT[(j, qsub, p)],
                       wp_sb[:, C * p + NQ * n:C * p + NQ * (n + 1)],
                       start=(p == 0), stop=(p == 1))
                yt = stage.tile([128, NQ], bf16, tag="y", bufs=4,
                                name=f"yt{tt}{n}")
                nc.vector.tensor_copy(yt, yps)
                nc.sync.dma_start(
                    out=y_d[128 * tt:128 * (tt + 1), NQ * n:NQ * (n + 1)],
                    in_=yt)
            return go

        ctxT = {}

        def force(tag):
            # emit queued chunks with this tag now (deps require them)
            if any(t == tag for t, _ in rest_q):
                keep = [e for e in rest_q if e[0] != tag]
                run = [fn for t, fn in rest_q if t == tag]
                rest_q.clear()
                rest_q.extend(keep)
                for fn in run:
                    fn()

        # QKV block 0: q/k emitted directly (scores j0 need them); v
        # chains staggered into the j0 loop right before each pv chain.
        for tag, ch in qkv_chunks(0, split_qk=True):
            if tag.startswith("v"):
                rest_q.append((tag, ch))
            else:
                ch()

        # yproj of j0/j1 is deferred into the j2/j3 stream, where Act
        # (exp) is the bottleneck and PE otherwise idles on the score
        # psum WAR; ct bufs=32 keeps ctxT alive across the deferral.
        defer_q = deque()

        kv_next = []
        for j in range(NTQ):
            force(f"q{j}")   # scores(j) stationary needs qT block j
            if j + 1 < NTQ:
                ch = qkv_chunks(j + 1)
                if j == 1:
                    # q2 gates j2 scores -> drain during j1; k2/v2 are
                    # first needed at j2's diagonal (tk8) -> fill j2's
                    # early pump slots instead of PE-bound j1
                    rest_q.extend(ch[:2])
                    kv_next = ch[2:]
                else:
                    rest_q.extend(ch)
            if j == 2 and kv_next:
                rest_q.extend(kv_next)
                kv_next = []
            if j == 3:
                # deferred j0/j1 yproj fills j3, where Act has the most
                # surplus and PE otherwise stalls on the score-psum WAR
                rest_q.extend(defer_q)
                defer_q.clear()
            ntk = 4 * j + 4
            for tk in range(ntk):
                di = tk - 4 * j
                if di == 0:
                    force(f"k{j}")   # diagonal scores need kT block j
                # p0/p1 interleaved per tk so pv chains can fire at their
                # diagonal tk (tail work weaves into the score stream, and
                # the pT WAR with next j's exps resolves naturally).
                scores_exp_mask(j, 0, tk)
                scores_exp_mask(j, 1, tk)
                if di >= 0:
                    force(f"v{j}t{tk}")  # pv chain reads vt tiles up to tk
                    pv_chunk(j, di)()
                    yq = defer_q if j < 2 else rest_q
                    for n in range(2):
                        yq.append((f"y{j}", yproj_chunk(j, di, n)))
                    pump(3 if j < 3 else 2)
                else:
                    pump(3 if j < 3 else 2)
        pump(10 ** 6)

        dump("qT0", qT[0], stage)
        dump("kT0", kTz[0][0], stage)
        dump("v0", vt[0], stage)
        dump("pT00", pT[0][0], stage)
        dump("pT03", pT[0][3], stage)


def build_nc(debug_names=None):
    import concourse.mybir as mybir
    import concourse.tile as tile
    from concourse import bacc

    f32 = mybir.dt.float32
    bf16 = mybir.dt.bfloat16
    nc = bacc.Bacc("TRN2", target_bir_lowering=False, debug=False)
    # all inputs pre-arranged host-side into the exact SBUF layouts:
    #   xT:  [128, (tq, c, t)]  t-tile-major x
    #   wT:  [128, (c, d)]      per-c-slice weight tiles
    #   wpT: [128, (s, c)]
    aps = (
        nc.dram_tensor("xT", [128, NCSL * T], bf16, kind="ExternalInput").ap(),
        nc.dram_tensor("wqT", [128, NCSL * DLOC], bf16, kind="ExternalInput").ap(),
        nc.dram_tensor("wkT", [128, NCSL * DLOC], bf16, kind="ExternalInput").ap(),
        nc.dram_tensor("wvT", [128, NCSL * DLOC], bf16, kind="ExternalInput").ap(),
        nc.dram_tensor("wpT", [128, 2 * C], bf16, kind="ExternalInput").ap(),
        nc.dram_tensor("mask2", [128, 256], bf16, kind="ExternalInput").ap(),
        nc.dram_tensor("ident", [128, 128], bf16, kind="ExternalInput").ap(),
        nc.dram_tensor("y", [T, C], bf16, kind="ExternalOutput").ap(),
    )
    dbg = None
    if debug_names:
        dbg = {}
        for name, shape in debug_names.items():
            dbg[name] = nc.dram_tensor(
                f"dbg_{name}", list(shape), f32, kind="ExternalOutput").ap()
    with tile.TileContext(nc) as tc:
        _emit(nc, tc, aps, dbg=dbg)
    nc.compile()
    return nc


def get_nc():
    global _NC_CACHE
    if _NC_CACHE is None:
        _NC_CACHE = build_nc()
    return _NC_CACHE


def prepare_in_maps(x, Wk, Wq, Wv, Wp):
    import ml_dtypes

    bf = ml_dtypes.bfloat16
    x = np.asarray(x, np.float32)
    Wk, Wq, Wv, Wp = (np.asarray(w, np.float32) for w in (Wk, Wq, Wv, Wp))
    tri = (np.arange(128)[:, None] <= np.arange(128)[None, :])
    mask2 = np.concatenate([tri, tri], axis=1).astype(bf)
    ident = np.eye(128, dtype=bf)
    scale = np.float32(1.0 / np.sqrt(np.float32(D)))

    def x_sbuf(xb):
        # xT [C, T] -> [128 p, (tq, c, t)]: partition p holds channel
        # 128*c + p, t-tile-major free layout
        xT = xb.T.reshape(NCSL, 128, NT128, 128)      # [c, p, tq, t]
        return np.ascontiguousarray(
            xT.transpose(1, 2, 0, 3).reshape(128, NCSL * T)).astype(bf)

    def w_sbuf(wT):
        # wT [C, DLOC] -> [128 p, (c, d)]
        w = wT.reshape(NCSL, 128, DLOC)               # [c, p, d]
        return np.ascontiguousarray(
            w.transpose(1, 0, 2).reshape(128, NCSL * DLOC)).astype(bf)

    in_maps = []
    for core in range(N_CORES):
        b, g = core // 4, core % 4
        dl = slice(DLOC * g, DLOC * (g + 1))
        wpT = Wp[:, dl].T                              # [DLOC, C]
        wp_s = np.ascontiguousarray(
            wpT.reshape(2, 128, C).transpose(1, 0, 2).reshape(128, 2 * C)
        ).astype(bf)
        in_maps.append({
            "xT": x_sbuf(x[b]),
            "wqT": w_sbuf((Wq[dl, :] * scale).T),
            "wkT": w_sbuf(Wk[dl, :].T),
            "wvT": w_sbuf(Wv[dl, :].T),
            "wpT": wp_s,
            "mask2": mask2,
            "ident": ident,
        })
    return in_maps


def combine_results(results):
    y = np.zeros((B, T, C), np.float32)
    for core in range(N_CORES):
        y[core // 4] += np.asarray(results[core]["y"], np.float32)
    return y


def kernel(**inputs):
    from concourse.bass_utils import run_bass_kernel_spmd

    nc = get_nc()
    in_maps = prepare_in_maps(
        inputs["x"], inputs["Wk"], inputs["Wq"], inputs["Wv"], inputs["Wp"])
    res = run_bass_kernel_spmd(nc, in_maps, list(range(N_CORES)))
    return combine_results(res.results)

